# revision 5
# baseline (speedup 1.0000x reference)
"""Trainium2 Bass kernel for nn_ARIMA_59373627900094 (flow-sampling ARIMA MLP).

Math: 100 Euler steps of x <- x + dt*(MLP([x,t]) - noise), x0 = noise,
over B*C = 16384 independent rows of dim 97 (MLP: 98 -> 100 -> 100 -> 97, tanh).

Device formulation (per row, feature-major) avoids materializing x entirely:
  z~_i = W1x @ x_i - i*v   kept in PSUM (accumulating matmuls)
  h1 = tanh(z~_i + btab[:,i])            # btab folds b1, the time input, and i*v
  h2 = tanh(W2 @ h1 + b2)
  z~_{i+1} = z~_i + G @ h2 - dt*(W1x @ noise)   # two accumulating matmuls
  S += h2                                 # VectorE
  out = dt*W3 @ S + b3                    # exact: 100*dt*noise cancels x0

Sharding: pure data parallel, batch dim across 8 cores (2048 rows each).
"""

import sys

for _p in ("/opt/trn_rl_repo",):
    if _p not in sys.path:
        sys.path.insert(0, _p)

import numpy as np

B, Q, C, H, NSTEPS = 1024, 96, 16, 100, 100
NCORES = 8
FEAT = Q + 1          # 97
ROWS = B * C          # 16384
RPC = ROWS // NCORES  # 2048 rows per core
NCHUNK = 2
CHUNK = RPC // NCHUNK  # 1024
MMF = 512             # matmul free dim (one PSUM bank of fp32)

_COMPILED = {}


def _build():
    import concourse.bacc as bacc
    import concourse.bass as bass
    import concourse.tile as tile

    mybir = bass.mybir
    f32 = mybir.dt.float32
    f32r = mybir.dt.float32r
    Tanh = mybir.ActivationFunctionType.Tanh
    add = mybir.AluOpType.add

    nc = bacc.Bacc("TRN2", target_bir_lowering=False, debug=False,
                   num_devices=NCORES)

    noise_ext = nc.declare_dram_parameter("noise", [FEAT, RPC], f32r, isOutput=False)
    w1xT_ext = nc.declare_dram_parameter("w1xT", [FEAT, H], f32r, isOutput=False)
    w2T_ext = nc.declare_dram_parameter("w2T", [H, H], f32r, isOutput=False)
    gT_ext = nc.declare_dram_parameter("gT", [H, H], f32r, isOutput=False)
    cT_ext = nc.declare_dram_parameter("cT", [FEAT, H], f32r, isOutput=False)
    w3dT_ext = nc.declare_dram_parameter("w3dT", [H, FEAT], f32r, isOutput=False)
    btab_ext = nc.declare_dram_parameter("btab", [H, NSTEPS], f32, isOutput=False)
    b2_ext = nc.declare_dram_parameter("b2c", [H, 1], f32, isOutput=False)
    b3_ext = nc.declare_dram_parameter("b3c", [FEAT, 1], f32, isOutput=False)
    out_ext = nc.declare_dram_parameter("out", [FEAT, RPC], f32, isOutput=True)

    with tile.TileContext(nc) as tc:
        with tc.tile_pool(name="const", bufs=1) as cp, \
             tc.tile_pool(name="work", bufs=3) as wp, \
             tc.tile_pool(name="zp", bufs=1, space="PSUM") as zp, \
             tc.tile_pool(name="mp", bufs=1, space="PSUM") as mp:

            n_sb = cp.tile([FEAT, RPC], f32r, tag="n")
            w1xT = cp.tile([FEAT, H], f32r, tag="w1xT")
            w2T = cp.tile([H, H], f32r, tag="w2T")
            gT = cp.tile([H, H], f32r, tag="gT")
            cT = cp.tile([FEAT, H], f32r, tag="cT")
            w3dT = cp.tile([H, FEAT], f32r, tag="w3dT")
            btab = cp.tile([H, NSTEPS], f32, tag="btab")
            b2c = cp.tile([H, 1], f32, tag="b2c")
            b3c = cp.tile([FEAT, 1], f32, tag="b3c")
            S = cp.tile([H, RPC], f32r, tag="S")

            for t, e in ((n_sb, noise_ext), (w1xT, w1xT_ext), (w2T, w2T_ext),
                         (gT, gT_ext), (cT, cT_ext), (w3dT, w3dT_ext),
                         (btab, btab_ext), (b2c, b2_ext), (b3c, b3_ext)):
                nc.sync.dma_start(out=t[:], in_=e[:])

            # z~_0 = W1x @ noise, one persistent 2-bank PSUM tile per chunk
            z = []
            for ch in range(NCHUNK):
                zt = zp.tile([H, CHUNK], f32, tag=f"z{ch}")
                z.append(zt)
                for s in range(CHUNK // MMF):
                    col = ch * CHUNK + s * MMF
                    nc.tensor.matmul(
                        zt[:, s * MMF:(s + 1) * MMF],
                        lhsT=w1xT[:],
                        rhs=n_sb[:, col:col + MMF],
                        start=True, stop=False)

            for i in range(NSTEPS):
                for ch in range(NCHUNK):
                    c0 = ch * CHUNK
                    h1 = wp.tile([H, CHUNK], f32r, tag=f"h1_{ch}")
                    nc.scalar.activation(h1[:], z[ch][:], Tanh,
                                         bias=btab[:, i:i + 1], scale=1.0)
                    ps2 = mp.tile([H, CHUNK], f32, tag=f"ps2_{ch}")
                    for s in range(CHUNK // MMF):
                        sl = slice(s * MMF, (s + 1) * MMF)
                        nc.tensor.matmul(ps2[:, sl], lhsT=w2T[:],
                                         rhs=h1[:, sl], start=True, stop=True)
                    h2 = wp.tile([H, CHUNK], f32r, tag=f"h2_{ch}")
                    nc.scalar.activation(h2[:], ps2[:], Tanh,
                                         bias=b2c[:, 0:1], scale=1.0)
                    if i == 0:
                        nc.vector.tensor_copy(S[:, c0:c0 + CHUNK], h2[:])
                    else:
                        nc.vector.tensor_tensor(S[:, c0:c0 + CHUNK],
                                                S[:, c0:c0 + CHUNK], h2[:], add)
                    if i < NSTEPS - 1:
                        for s in range(CHUNK // MMF):
                            sl = slice(s * MMF, (s + 1) * MMF)
                            col = c0 + s * MMF
                            nc.tensor.matmul(z[ch][:, sl], lhsT=gT[:],
                                             rhs=h2[:, sl],
                                             start=False, stop=False)
                            nc.tensor.matmul(z[ch][:, sl], lhsT=cT[:],
                                             rhs=n_sb[:, col:col + MMF],
                                             start=False, stop=(i == NSTEPS - 2))

            # out = dt*W3 @ S + b3
            for ch in range(NCHUNK):
                c0 = ch * CHUNK
                pO = mp.tile([FEAT, CHUNK], f32, tag=f"ps2_{ch}")
                for s in range(CHUNK // MMF):
                    sl = slice(s * MMF, (s + 1) * MMF)
                    nc.tensor.matmul(pO[:, sl], lhsT=w3dT[:],
                                     rhs=S[:, c0 + s * MMF:c0 + (s + 1) * MMF],
                                     start=True, stop=True)
                o_sb = wp.tile([FEAT, CHUNK], f32, tag=f"o_{ch}")
                nc.vector.tensor_scalar_add(o_sb[:], pO[:], b3c[:, 0:1])
                nc.sync.dma_start(out=out_ext[:, c0:c0 + CHUNK], in_=o_sb[:])

    nc.compile()
    return nc


def _get_nc():
    if "nc" not in _COMPILED:
        _COMPILED["nc"] = _build()
    return _COMPILED["nc"]


def _host_prep(series, rand_error, W1, b1, W2, b2, W3, b3):
    dt = np.float32(1.0 / NSTEPS)
    noise = np.concatenate([series, rand_error], axis=1)        # (B, 97, C)
    n = np.ascontiguousarray(
        noise.transpose(1, 0, 2).reshape(FEAT, ROWS), np.float32)  # (97, rows)

    W1x = W1[:, :FEAT]                                          # (100, 97)
    w1t = W1[:, FEAT]                                           # (100,)
    v = dt * (W1x @ b3)                                         # (100,)
    steps = np.arange(NSTEPS, dtype=np.float32)
    btab = (b1[:, None] + np.outer(w1t, steps / NSTEPS)
            + np.outer(v, steps)).astype(np.float32)            # (100, 100)

    shared = {
        "w1xT": np.ascontiguousarray(W1x.T, np.float32),
        "w2T": np.ascontiguousarray(W2.T, np.float32),
        "gT": np.ascontiguousarray((dt * (W1x @ W3)).T, np.float32),
        "cT": np.ascontiguousarray((-dt * W1x).T, np.float32),
        "w3dT": np.ascontiguousarray((dt * W3).T, np.float32),
        "btab": np.ascontiguousarray(btab, np.float32),
        "b2c": np.ascontiguousarray(b2[:, None], np.float32),
        "b3c": np.ascontiguousarray(b3[:, None], np.float32),
    }
    in_maps = []
    for core in range(NCORES):
        m = dict(shared)
        m["noise"] = np.ascontiguousarray(n[:, core * RPC:(core + 1) * RPC])
        in_maps.append(m)
    return in_maps


def kernel(series, rand_error, W1, b1, W2, b2, W3, b3, _trace=False,
           _tmpdir=None, _nc_out=None):
    from concourse.bass_utils import run_bass_kernel_spmd

    args = [np.asarray(a, np.float32) for a in
            (series, rand_error, W1, b1, W2, b2, W3, b3)]
    in_maps = _host_prep(*args)
    nc = _get_nc()
    if _nc_out is not None:
        _nc_out.append(nc)
    res = run_bass_kernel_spmd(nc, in_maps, core_ids=list(range(NCORES)),
                               trace=_trace, tmpdir=_tmpdir)
    outs = [np.asarray(res.results[i]["out"]) for i in range(NCORES)]
    full = np.concatenate(outs, axis=1)                         # (97, rows)
    out = full.reshape(FEAT, B, C).transpose(1, 0, 2)           # (B, 97, C)
    if _trace:
        return np.ascontiguousarray(out), res
    return np.ascontiguousarray(out)


# revision 6
# speedup vs baseline: 1.0603x; 1.0603x over previous
"""Trainium2 Bass kernel for nn_ARIMA_59373627900094 (flow-sampling ARIMA MLP).

Math: 100 Euler steps of x <- x + dt*(MLP([x,t]) - noise), x0 = noise,
over B*C = 16384 independent rows of dim 97 (MLP: 98 -> 100 -> 100 -> 97, tanh).

Device formulation (per row, feature-major) avoids materializing x entirely:
  z~_i = W1x @ x_i - i*v   kept in PSUM (accumulating matmuls)
  h1 = tanh(z~_i + btab[:,i])            # btab folds b1, the time input, and i*v
  h2 = tanh(W2 @ h1 + b2)
  z~_{i+1} = z~_i + G @ h2 - dt*(W1x @ noise)   # two accumulating matmuls
  S += h2                                 # VectorE
  out = dt*W3 @ S + b3                    # exact: 100*dt*noise cancels x0

Sharding: pure data parallel, batch dim across 8 cores (2048 rows each).
"""

import sys

for _p in ("/opt/trn_rl_repo",):
    if _p not in sys.path:
        sys.path.insert(0, _p)

import numpy as np

B, Q, C, H, NSTEPS = 1024, 96, 16, 100, 100
NCORES = 8
FEAT = Q + 1          # 97
ROWS = B * C          # 16384
RPC = ROWS // NCORES  # 2048 rows per core
NCHUNK = 2
CHUNK = RPC // NCHUNK  # 1024
MMF = 512             # matmul free dim (one PSUM bank of fp32)

_COMPILED = {}


def _build():
    import concourse.bacc as bacc
    import concourse.bass as bass
    import concourse.tile as tile

    mybir = bass.mybir
    f32 = mybir.dt.float32
    f32r = mybir.dt.float32r
    Tanh = mybir.ActivationFunctionType.Tanh
    add = mybir.AluOpType.add

    nc = bacc.Bacc("TRN2", target_bir_lowering=False, debug=False,
                   num_devices=NCORES)

    bf16 = mybir.dt.bfloat16
    noise_ext = nc.declare_dram_parameter("noise", [FEAT, RPC], bf16, isOutput=False)
    w1xT_ext = nc.declare_dram_parameter("w1xT", [FEAT, H], bf16, isOutput=False)
    w2T_ext = nc.declare_dram_parameter("w2T", [H, H], bf16, isOutput=False)
    gT_ext = nc.declare_dram_parameter("gT", [H, H], bf16, isOutput=False)
    cT_ext = nc.declare_dram_parameter("cT", [FEAT, H], bf16, isOutput=False)
    w3dT_ext = nc.declare_dram_parameter("w3dT", [H, FEAT], f32r, isOutput=False)
    btab_ext = nc.declare_dram_parameter("btab", [H, NSTEPS], f32, isOutput=False)
    b2_ext = nc.declare_dram_parameter("b2c", [H, 1], f32, isOutput=False)
    b3_ext = nc.declare_dram_parameter("b3c", [FEAT, 1], f32, isOutput=False)
    out_ext = nc.declare_dram_parameter("out", [FEAT, RPC], f32, isOutput=True)

    with tile.TileContext(nc) as tc:
        with tc.tile_pool(name="const", bufs=1) as cp, \
             tc.tile_pool(name="work", bufs=3) as wp, \
             tc.tile_pool(name="zp", bufs=1, space="PSUM") as zp, \
             tc.tile_pool(name="mp", bufs=1, space="PSUM") as mp:

            n_sb = cp.tile([FEAT, RPC], bf16, tag="n")
            w1xT = cp.tile([FEAT, H], bf16, tag="w1xT")
            w2T = cp.tile([H, H], bf16, tag="w2T")
            gT = cp.tile([H, H], bf16, tag="gT")
            cT = cp.tile([FEAT, H], bf16, tag="cT")
            w3dT = cp.tile([H, FEAT], f32r, tag="w3dT")
            btab = cp.tile([H, NSTEPS], f32, tag="btab")
            b2c = cp.tile([H, 1], f32, tag="b2c")
            b3c = cp.tile([FEAT, 1], f32, tag="b3c")
            S = cp.tile([H, RPC], f32r, tag="S")

            for t, e in ((n_sb, noise_ext), (w1xT, w1xT_ext), (w2T, w2T_ext),
                         (gT, gT_ext), (cT, cT_ext), (w3dT, w3dT_ext),
                         (btab, btab_ext), (b2c, b2_ext), (b3c, b3_ext)):
                nc.sync.dma_start(out=t[:], in_=e[:])

            # z~_0 = W1x @ noise, one persistent 2-bank PSUM tile per chunk
            z = []
            for ch in range(NCHUNK):
                zt = zp.tile([H, CHUNK], f32, tag=f"z{ch}")
                z.append(zt)
                for s in range(CHUNK // MMF):
                    col = ch * CHUNK + s * MMF
                    nc.tensor.matmul(
                        zt[:, s * MMF:(s + 1) * MMF],
                        lhsT=w1xT[:],
                        rhs=n_sb[:, col:col + MMF],
                        start=True, stop=False)

            for i in range(NSTEPS):
                for ch in range(NCHUNK):
                    c0 = ch * CHUNK
                    h1 = wp.tile([H, CHUNK], bf16, tag=f"h1_{ch}")
                    nc.scalar.activation(h1[:], z[ch][:], Tanh,
                                         bias=btab[:, i:i + 1], scale=1.0)
                    ps2 = mp.tile([H, CHUNK], f32, tag=f"ps2_{ch}")
                    for s in range(CHUNK // MMF):
                        sl = slice(s * MMF, (s + 1) * MMF)
                        nc.tensor.matmul(ps2[:, sl], lhsT=w2T[:],
                                         rhs=h1[:, sl], start=True, stop=True)
                    h2 = wp.tile([H, CHUNK], bf16, tag=f"h2_{ch}")
                    nc.scalar.activation(h2[:], ps2[:], Tanh,
                                         bias=b2c[:, 0:1], scale=1.0)
                    if i == 0:
                        nc.vector.tensor_copy(S[:, c0:c0 + CHUNK], h2[:])
                    else:
                        nc.vector.tensor_tensor(S[:, c0:c0 + CHUNK],
                                                S[:, c0:c0 + CHUNK], h2[:], add)
                    if i < NSTEPS - 1:
                        for s in range(CHUNK // MMF):
                            sl = slice(s * MMF, (s + 1) * MMF)
                            col = c0 + s * MMF
                            nc.tensor.matmul(z[ch][:, sl], lhsT=gT[:],
                                             rhs=h2[:, sl],
                                             start=False, stop=False)
                            nc.tensor.matmul(z[ch][:, sl], lhsT=cT[:],
                                             rhs=n_sb[:, col:col + MMF],
                                             start=False, stop=(i == NSTEPS - 2))

            # out = dt*W3 @ S + b3
            for ch in range(NCHUNK):
                c0 = ch * CHUNK
                pO = mp.tile([FEAT, CHUNK], f32, tag=f"ps2_{ch}")
                for s in range(CHUNK // MMF):
                    sl = slice(s * MMF, (s + 1) * MMF)
                    nc.tensor.matmul(pO[:, sl], lhsT=w3dT[:],
                                     rhs=S[:, c0 + s * MMF:c0 + (s + 1) * MMF],
                                     start=True, stop=True)
                o_sb = wp.tile([FEAT, CHUNK], f32, tag=f"o_{ch}")
                nc.vector.tensor_scalar_add(o_sb[:], pO[:], b3c[:, 0:1])
                nc.sync.dma_start(out=out_ext[:, c0:c0 + CHUNK], in_=o_sb[:])

    nc.compile()
    return nc


def _get_nc():
    if "nc" not in _COMPILED:
        _COMPILED["nc"] = _build()
    return _COMPILED["nc"]


def _host_prep(series, rand_error, W1, b1, W2, b2, W3, b3):
    dt = np.float32(1.0 / NSTEPS)
    noise = np.concatenate([series, rand_error], axis=1)        # (B, 97, C)
    n = np.ascontiguousarray(
        noise.transpose(1, 0, 2).reshape(FEAT, ROWS), np.float32)  # (97, rows)

    W1x = W1[:, :FEAT]                                          # (100, 97)
    w1t = W1[:, FEAT]                                           # (100,)
    v = dt * (W1x @ b3)                                         # (100,)
    steps = np.arange(NSTEPS, dtype=np.float32)
    btab = (b1[:, None] + np.outer(w1t, steps / NSTEPS)
            + np.outer(v, steps)).astype(np.float32)            # (100, 100)

    import ml_dtypes
    bf16 = ml_dtypes.bfloat16
    shared = {
        "w1xT": np.ascontiguousarray(W1x.T.astype(bf16)),
        "w2T": np.ascontiguousarray(W2.T.astype(bf16)),
        "gT": np.ascontiguousarray((dt * (W1x @ W3)).T.astype(bf16)),
        "cT": np.ascontiguousarray((-dt * W1x).T.astype(bf16)),
        "w3dT": np.ascontiguousarray((dt * W3).T, np.float32),
        "btab": np.ascontiguousarray(btab, np.float32),
        "b2c": np.ascontiguousarray(b2[:, None], np.float32),
        "b3c": np.ascontiguousarray(b3[:, None], np.float32),
    }
    in_maps = []
    for core in range(NCORES):
        m = dict(shared)
        m["noise"] = np.ascontiguousarray(n[:, core * RPC:(core + 1) * RPC].astype(bf16))
        in_maps.append(m)
    return in_maps


def kernel(series, rand_error, W1, b1, W2, b2, W3, b3, _trace=False,
           _tmpdir=None, _nc_out=None):
    from concourse.bass_utils import run_bass_kernel_spmd

    args = [np.asarray(a, np.float32) for a in
            (series, rand_error, W1, b1, W2, b2, W3, b3)]
    in_maps = _host_prep(*args)
    nc = _get_nc()
    if _nc_out is not None:
        _nc_out.append(nc)
    res = run_bass_kernel_spmd(nc, in_maps, core_ids=list(range(NCORES)),
                               trace=_trace, tmpdir=_tmpdir)
    outs = [np.asarray(res.results[i]["out"]) for i in range(NCORES)]
    full = np.concatenate(outs, axis=1)                         # (97, rows)
    out = full.reshape(FEAT, B, C).transpose(1, 0, 2)           # (B, 97, C)
    if _trace:
        return np.ascontiguousarray(out), res
    return np.ascontiguousarray(out)


# revision 8
# speedup vs baseline: 1.1535x; 1.0879x over previous
"""Trainium2 Bass kernel for nn_ARIMA_59373627900094 (flow-sampling ARIMA MLP).

Math: 100 Euler steps of x <- x + dt*(MLP([x,t]) - noise), x0 = noise,
over B*C = 16384 independent rows of dim 97 (MLP: 98 -> 100 -> 100 -> 97, tanh).

Device formulation (per row, feature-major) avoids materializing x entirely:
  z~_i = W1x @ x_i - i*v   kept in PSUM (accumulating matmuls)
  h1 = tanh(z~_i + btab[:,i])            # btab folds b1, the time input, and i*v
  h2 = tanh(W2 @ h1 + b2)
  z~_{i+1} = z~_i + G @ h2 - dt*(W1x @ noise)   # two accumulating matmuls
  S += h2                                 # VectorE
  out = dt*W3 @ S + b3                    # exact: 100*dt*noise cancels x0

Sharding: pure data parallel, batch dim across 8 cores (2048 rows each).
"""

import sys

for _p in ("/opt/trn_rl_repo",):
    if _p not in sys.path:
        sys.path.insert(0, _p)

import numpy as np

B, Q, C, H, NSTEPS = 1024, 96, 16, 100, 100
NCORES = 8
FEAT = Q + 1          # 97
ROWS = B * C          # 16384
RPC = ROWS // NCORES  # 2048 rows per core
NCHUNK = 2
CHUNK = RPC // NCHUNK  # 1024
MMF = 512             # matmul free dim (one PSUM bank of fp32)

_COMPILED = {}


def _build():
    import concourse.bacc as bacc
    import concourse.bass as bass
    import concourse.tile as tile

    mybir = bass.mybir
    f32 = mybir.dt.float32
    f32r = mybir.dt.float32r
    Tanh = mybir.ActivationFunctionType.Tanh
    add = mybir.AluOpType.add

    nc = bacc.Bacc("TRN2", target_bir_lowering=False, debug=False,
                   num_devices=NCORES)

    bf16 = mybir.dt.bfloat16
    noise_ext = nc.declare_dram_parameter("noise", [FEAT, RPC], bf16, isOutput=False)
    w1xT_ext = nc.declare_dram_parameter("w1xT", [FEAT, H], bf16, isOutput=False)
    w2T_ext = nc.declare_dram_parameter("w2T", [H, H], bf16, isOutput=False)
    gT_ext = nc.declare_dram_parameter("gT", [H, H], bf16, isOutput=False)
    cT_ext = nc.declare_dram_parameter("cT", [FEAT, H], bf16, isOutput=False)
    w3dT_ext = nc.declare_dram_parameter("w3dT", [H, FEAT], f32r, isOutput=False)
    btab_ext = nc.declare_dram_parameter("btab", [H, NSTEPS], f32, isOutput=False)
    b2_ext = nc.declare_dram_parameter("b2c", [H, 1], f32, isOutput=False)
    b3_ext = nc.declare_dram_parameter("b3c", [FEAT, 1], f32, isOutput=False)
    out_ext = nc.declare_dram_parameter("out", [FEAT, RPC], f32, isOutput=True)

    with tile.TileContext(nc) as tc:
        with tc.tile_pool(name="const", bufs=1) as cp, \
             tc.tile_pool(name="work", bufs=3) as wp, \
             tc.tile_pool(name="zp", bufs=1, space="PSUM") as zp, \
             tc.tile_pool(name="mp", bufs=1, space="PSUM") as mp:

            n_sb = cp.tile([FEAT, RPC], bf16, tag="n")
            w1xT = cp.tile([FEAT, H], bf16, tag="w1xT")
            w2T = cp.tile([H, H], bf16, tag="w2T")
            gT = cp.tile([H, H], bf16, tag="gT")
            cT = cp.tile([FEAT, H], bf16, tag="cT")
            w3dT = cp.tile([H, FEAT], f32r, tag="w3dT")
            btab = cp.tile([H, NSTEPS], f32, tag="btab")
            b2c = cp.tile([H, 1], f32, tag="b2c")
            b3c = cp.tile([FEAT, 1], f32, tag="b3c")
            S = cp.tile([H, RPC], f32r, tag="S")

            nc.gpsimd.dma_start(out=n_sb[:, 0:CHUNK], in_=noise_ext[:, 0:CHUNK])
            nc.scalar.dma_start(out=n_sb[:, CHUNK:RPC],
                                in_=noise_ext[:, CHUNK:RPC])
            for t, e in ((w1xT, w1xT_ext), (btab, btab_ext), (w2T, w2T_ext),
                         (gT, gT_ext), (cT, cT_ext), (b2c, b2_ext),
                         (w3dT, w3dT_ext), (b3c, b3_ext)):
                nc.sync.dma_start(out=t[:], in_=e[:])

            # z~_0 = W1x @ noise, one persistent 2-bank PSUM tile per chunk
            z = []
            for ch in range(NCHUNK):
                zt = zp.tile([H, CHUNK], f32, tag=f"z{ch}")
                z.append(zt)
                for s in range(CHUNK // MMF):
                    col = ch * CHUNK + s * MMF
                    nc.tensor.matmul(
                        zt[:, s * MMF:(s + 1) * MMF],
                        lhsT=w1xT[:],
                        rhs=n_sb[:, col:col + MMF],
                        start=True, stop=False)

            for i in range(NSTEPS):
                for ch in range(NCHUNK):
                    c0 = ch * CHUNK
                    h1 = wp.tile([H, CHUNK], bf16, tag=f"h1_{ch}")
                    nc.scalar.activation(h1[:], z[ch][:], Tanh,
                                         bias=btab[:, i:i + 1], scale=1.0)
                    ps2 = mp.tile([H, CHUNK], f32, tag=f"ps2_{ch}")
                    for s in range(CHUNK // MMF):
                        sl = slice(s * MMF, (s + 1) * MMF)
                        nc.tensor.matmul(ps2[:, sl], lhsT=w2T[:],
                                         rhs=h1[:, sl], start=True, stop=True)
                    if i < NSTEPS - 1:
                        for s in range(CHUNK // MMF):
                            sl = slice(s * MMF, (s + 1) * MMF)
                            col = c0 + s * MMF
                            nc.tensor.matmul(z[ch][:, sl], lhsT=cT[:],
                                             rhs=n_sb[:, col:col + MMF],
                                             start=False, stop=False)
                    h2 = wp.tile([H, CHUNK], bf16, tag=f"h2_{ch}")
                    nc.scalar.activation(h2[:], ps2[:], Tanh,
                                         bias=b2c[:, 0:1], scale=1.0)
                    if i == 0:
                        nc.vector.tensor_copy(S[:, c0:c0 + CHUNK], h2[:])
                    else:
                        nc.vector.tensor_tensor(S[:, c0:c0 + CHUNK],
                                                S[:, c0:c0 + CHUNK], h2[:], add)
                    if i < NSTEPS - 1:
                        for s in range(CHUNK // MMF):
                            sl = slice(s * MMF, (s + 1) * MMF)
                            nc.tensor.matmul(z[ch][:, sl], lhsT=gT[:],
                                             rhs=h2[:, sl],
                                             start=False, stop=(i == NSTEPS - 2))

            # out = dt*W3 @ S + b3
            for ch in range(NCHUNK):
                c0 = ch * CHUNK
                pO = mp.tile([FEAT, CHUNK], f32, tag=f"ps2_{ch}")
                for s in range(CHUNK // MMF):
                    sl = slice(s * MMF, (s + 1) * MMF)
                    nc.tensor.matmul(pO[:, sl], lhsT=w3dT[:],
                                     rhs=S[:, c0 + s * MMF:c0 + (s + 1) * MMF],
                                     start=True, stop=True)
                o_sb = wp.tile([FEAT, CHUNK], f32, tag=f"o_{ch}")
                nc.vector.tensor_scalar_add(o_sb[:], pO[:], b3c[:, 0:1])
                nc.gpsimd.dma_start(out=out_ext[:, c0:c0 + CHUNK], in_=o_sb[:])

    nc.compile()
    return nc


def _get_nc():
    if "nc" not in _COMPILED:
        _COMPILED["nc"] = _build()
    return _COMPILED["nc"]


def _host_prep(series, rand_error, W1, b1, W2, b2, W3, b3):
    dt = np.float32(1.0 / NSTEPS)
    noise = np.concatenate([series, rand_error], axis=1)        # (B, 97, C)
    n = np.ascontiguousarray(
        noise.transpose(1, 0, 2).reshape(FEAT, ROWS), np.float32)  # (97, rows)

    W1x = W1[:, :FEAT]                                          # (100, 97)
    w1t = W1[:, FEAT]                                           # (100,)
    v = dt * (W1x @ b3)                                         # (100,)
    steps = np.arange(NSTEPS, dtype=np.float32)
    btab = (b1[:, None] + np.outer(w1t, steps / NSTEPS)
            + np.outer(v, steps)).astype(np.float32)            # (100, 100)

    import ml_dtypes
    bf16 = ml_dtypes.bfloat16
    shared = {
        "w1xT": np.ascontiguousarray(W1x.T.astype(bf16)),
        "w2T": np.ascontiguousarray(W2.T.astype(bf16)),
        "gT": np.ascontiguousarray((dt * (W1x @ W3)).T.astype(bf16)),
        "cT": np.ascontiguousarray((-dt * W1x).T.astype(bf16)),
        "w3dT": np.ascontiguousarray((dt * W3).T, np.float32),
        "btab": np.ascontiguousarray(btab, np.float32),
        "b2c": np.ascontiguousarray(b2[:, None], np.float32),
        "b3c": np.ascontiguousarray(b3[:, None], np.float32),
    }
    in_maps = []
    for core in range(NCORES):
        m = dict(shared)
        m["noise"] = np.ascontiguousarray(n[:, core * RPC:(core + 1) * RPC].astype(bf16))
        in_maps.append(m)
    return in_maps


def kernel(series, rand_error, W1, b1, W2, b2, W3, b3, _trace=False,
           _tmpdir=None, _nc_out=None):
    from concourse.bass_utils import run_bass_kernel_spmd

    args = [np.asarray(a, np.float32) for a in
            (series, rand_error, W1, b1, W2, b2, W3, b3)]
    in_maps = _host_prep(*args)
    nc = _get_nc()
    if _nc_out is not None:
        _nc_out.append(nc)
    res = run_bass_kernel_spmd(nc, in_maps, core_ids=list(range(NCORES)),
                               trace=_trace, tmpdir=_tmpdir)
    outs = [np.asarray(res.results[i]["out"]) for i in range(NCORES)]
    full = np.concatenate(outs, axis=1)                         # (97, rows)
    out = full.reshape(FEAT, B, C).transpose(1, 0, 2)           # (B, 97, C)
    if _trace:
        return np.ascontiguousarray(out), res
    return np.ascontiguousarray(out)


# revision 10
# speedup vs baseline: 1.3658x; 1.1840x over previous
"""Trainium2 Bass kernel for nn_ARIMA_59373627900094 (flow-sampling ARIMA MLP).

Math: 100 Euler steps of x <- x + dt*(MLP([x,t]) - noise), x0 = noise,
over B*C = 16384 independent rows of dim 97 (MLP: 98 -> 100 -> 100 -> 97, tanh).

Device formulation (per row, feature-major) avoids materializing x entirely:
  z~_i = W1x @ x_i - i*v   kept in PSUM (accumulating matmuls)
  h1 = tanh(z~_i + btab[:,i])            # btab folds b1, the time input, and i*v
  h2 = tanh(W2 @ h1 + b2)
  z~_{i+1} = z~_i + G @ h2 - dt*(W1x @ noise)   # two accumulating matmuls
  S += h2                                 # VectorE
  out = dt*W3 @ S + b3                    # exact: 100*dt*noise cancels x0

Sharding: pure data parallel, batch dim across 8 cores (2048 rows each).
"""

import sys

for _p in ("/opt/trn_rl_repo",):
    if _p not in sys.path:
        sys.path.insert(0, _p)

import numpy as np

B, Q, C, H, NSTEPS = 1024, 96, 16, 100, 100
NCORES = 8
FEAT = Q + 1          # 97
ROWS = B * C          # 16384
RPC = ROWS // NCORES  # 2048 rows per core
NCHUNK = 2
CHUNK = RPC // NCHUNK  # 1024
MMF = 512             # matmul free dim (one PSUM bank of fp32)

_COMPILED = {}


def _build():
    import concourse.bacc as bacc
    import concourse.bass as bass
    import concourse.tile as tile

    mybir = bass.mybir
    f32 = mybir.dt.float32
    f32r = mybir.dt.float32r
    Tanh = mybir.ActivationFunctionType.Tanh
    add = mybir.AluOpType.add

    nc = bacc.Bacc("TRN2", target_bir_lowering=False, debug=False,
                   num_devices=NCORES)

    bf16 = mybir.dt.bfloat16
    noise_ext = nc.declare_dram_parameter("noise", [FEAT, RPC], bf16, isOutput=False)
    w1xT_ext = nc.declare_dram_parameter("w1xT", [FEAT, H], bf16, isOutput=False)
    w2T_ext = nc.declare_dram_parameter("w2T", [H, H], bf16, isOutput=False)
    gT_ext = nc.declare_dram_parameter("gT", [H, H], bf16, isOutput=False)
    cT_ext = nc.declare_dram_parameter("cT", [FEAT, H], bf16, isOutput=False)
    w3dT_ext = nc.declare_dram_parameter("w3dT", [H, FEAT], f32r, isOutput=False)
    btab_ext = nc.declare_dram_parameter("btab", [H, NSTEPS + 2], f32, isOutput=False)
    out_ext = nc.declare_dram_parameter("out", [FEAT, RPC], f32, isOutput=True)

    with tile.TileContext(nc) as tc:
        with tc.tile_pool(name="const", bufs=1) as cp, \
             tc.tile_pool(name="work", bufs=3) as wp, \
             tc.tile_pool(name="zp", bufs=1, space="PSUM") as zp, \
             tc.tile_pool(name="mp", bufs=1, space="PSUM") as mp:

            n_sb = cp.tile([FEAT, RPC], bf16, tag="n")
            w1xT = cp.tile([FEAT, H], bf16, tag="w1xT")
            w2T = cp.tile([H, H], bf16, tag="w2T")
            gT = cp.tile([H, H], bf16, tag="gT")
            cT = cp.tile([FEAT, H], bf16, tag="cT")
            w3dT = cp.tile([H, FEAT], f32r, tag="w3dT")
            btab = cp.tile([H, NSTEPS + 2], f32, tag="btab")
            S = cp.tile([H, RPC], f32r, tag="S")

            nc.sync.dma_start(out=n_sb[:, 0:CHUNK], in_=noise_ext[:, 0:CHUNK])
            nc.sync.dma_start(out=w1xT[:], in_=w1xT_ext[:])
            nc.sync.dma_start(out=btab[:], in_=btab_ext[:])
            nc.gpsimd.dma_start(out=n_sb[:, CHUNK:RPC],
                                in_=noise_ext[:, CHUNK:RPC])
            nc.gpsimd.dma_start(out=w2T[:], in_=w2T_ext[:])
            nc.scalar.dma_start(out=gT[:], in_=gT_ext[:])
            nc.scalar.dma_start(out=cT[:], in_=cT_ext[:])
            nc.scalar.dma_start(out=w3dT[:], in_=w3dT_ext[:])
            scratch = nc.dram_tensor("scratch", [1, 102], f32)

            # z~_0 = W1x @ noise, one persistent 2-bank PSUM tile per chunk
            z = []
            for ch in range(NCHUNK):
                zt = zp.tile([H, CHUNK], f32, tag=f"z{ch}")
                z.append(zt)
                for s in range(CHUNK // MMF):
                    col = ch * CHUNK + s * MMF
                    nc.tensor.matmul(
                        zt[:, s * MMF:(s + 1) * MMF],
                        lhsT=w1xT[:],
                        rhs=n_sb[:, col:col + MMF],
                        start=True, stop=False)

            for i in range(NSTEPS):
                for ch in range(NCHUNK):
                    c0 = ch * CHUNK
                    h1 = wp.tile([H, CHUNK], bf16, tag=f"h1_{ch}")
                    nc.scalar.activation(h1[:], z[ch][:], Tanh,
                                         bias=btab[:, i:i + 1], scale=1.0)
                    ps2 = mp.tile([H, CHUNK], f32, tag=f"ps2_{ch}")
                    for s in range(CHUNK // MMF):
                        sl = slice(s * MMF, (s + 1) * MMF)
                        nc.tensor.matmul(ps2[:, sl], lhsT=w2T[:],
                                         rhs=h1[:, sl], start=True, stop=True)
                    if i < NSTEPS - 1:
                        for s in range(CHUNK // MMF):
                            sl = slice(s * MMF, (s + 1) * MMF)
                            col = c0 + s * MMF
                            nc.tensor.matmul(z[ch][:, sl], lhsT=cT[:],
                                             rhs=n_sb[:, col:col + MMF],
                                             start=False, stop=False)
                    h2 = wp.tile([H, CHUNK], bf16, tag=f"h2_{ch}")
                    nc.scalar.activation(h2[:], ps2[:], Tanh,
                                         bias=btab[:, NSTEPS:NSTEPS + 1], scale=1.0)
                    if i == NSTEPS - 10 and ch == 0:
                        nc.gpsimd.dma_start(out=scratch[0:1, :],
                                            in_=btab[0:1, 0:NSTEPS + 2])
                        nc.sync.dma_start(out=scratch[0:1, :],
                                          in_=btab[1:2, 0:NSTEPS + 2])
                    if i == 0:
                        nc.vector.tensor_copy(S[:, c0:c0 + CHUNK], h2[:])
                    else:
                        nc.vector.tensor_tensor(S[:, c0:c0 + CHUNK],
                                                S[:, c0:c0 + CHUNK], h2[:], add)
                    if i < NSTEPS - 1:
                        for s in range(CHUNK // MMF):
                            sl = slice(s * MMF, (s + 1) * MMF)
                            nc.tensor.matmul(z[ch][:, sl], lhsT=gT[:],
                                             rhs=h2[:, sl],
                                             start=False, stop=(i == NSTEPS - 2))

            # out = dt*W3 @ S + b3
            for ch in range(NCHUNK):
                c0 = ch * CHUNK
                pO = mp.tile([FEAT, CHUNK], f32, tag=f"ps2_{ch}")
                for s in range(CHUNK // MMF):
                    sl = slice(s * MMF, (s + 1) * MMF)
                    nc.tensor.matmul(pO[:, sl], lhsT=w3dT[:],
                                     rhs=S[:, c0 + s * MMF:c0 + (s + 1) * MMF],
                                     start=True, stop=True)
                o_sb = wp.tile([FEAT, CHUNK], f32, tag=f"o_{ch}")
                nc.vector.tensor_scalar_add(o_sb[:], pO[:], btab[:FEAT, NSTEPS + 1:NSTEPS + 2])
                (nc.gpsimd if ch == 0 else nc.sync).dma_start(out=out_ext[:, c0:c0 + CHUNK], in_=o_sb[:])

    nc.compile()
    return nc


def _get_nc():
    if "nc" not in _COMPILED:
        _COMPILED["nc"] = _build()
    return _COMPILED["nc"]


def _host_prep(series, rand_error, W1, b1, W2, b2, W3, b3):
    dt = np.float32(1.0 / NSTEPS)
    noise = np.concatenate([series, rand_error], axis=1)        # (B, 97, C)
    n = np.ascontiguousarray(
        noise.transpose(1, 0, 2).reshape(FEAT, ROWS), np.float32)  # (97, rows)

    W1x = W1[:, :FEAT]                                          # (100, 97)
    w1t = W1[:, FEAT]                                           # (100,)
    v = dt * (W1x @ b3)                                         # (100,)
    steps = np.arange(NSTEPS, dtype=np.float32)
    btab = (b1[:, None] + np.outer(w1t, steps / NSTEPS)
            + np.outer(v, steps)).astype(np.float32)            # (100, 100)
    b3p = np.zeros(H, np.float32)
    b3p[:FEAT] = b3
    btab = np.concatenate([btab, b2[:, None], b3p[:, None]], axis=1)  # (100, 102)

    import ml_dtypes
    bf16 = ml_dtypes.bfloat16
    shared = {
        "w1xT": np.ascontiguousarray(W1x.T.astype(bf16)),
        "w2T": np.ascontiguousarray(W2.T.astype(bf16)),
        "gT": np.ascontiguousarray((dt * (W1x @ W3)).T.astype(bf16)),
        "cT": np.ascontiguousarray((-dt * W1x).T.astype(bf16)),
        "w3dT": np.ascontiguousarray((dt * W3).T, np.float32),
        "btab": np.ascontiguousarray(btab, np.float32),
    }
    in_maps = []
    for core in range(NCORES):
        m = dict(shared)
        m["noise"] = np.ascontiguousarray(n[:, core * RPC:(core + 1) * RPC].astype(bf16))
        in_maps.append(m)
    return in_maps


def kernel(series, rand_error, W1, b1, W2, b2, W3, b3, _trace=False,
           _tmpdir=None, _nc_out=None):
    from concourse.bass_utils import run_bass_kernel_spmd

    args = [np.asarray(a, np.float32) for a in
            (series, rand_error, W1, b1, W2, b2, W3, b3)]
    in_maps = _host_prep(*args)
    nc = _get_nc()
    if _nc_out is not None:
        _nc_out.append(nc)
    res = run_bass_kernel_spmd(nc, in_maps, core_ids=list(range(NCORES)),
                               trace=_trace, tmpdir=_tmpdir)
    outs = [np.asarray(res.results[i]["out"]) for i in range(NCORES)]
    full = np.concatenate(outs, axis=1)                         # (97, rows)
    out = full.reshape(FEAT, B, C).transpose(1, 0, 2)           # (B, 97, C)
    if _trace:
        return np.ascontiguousarray(out), res
    return np.ascontiguousarray(out)
